# revision 11
# baseline (speedup 1.0000x reference)
"""Trainium2 Bass kernel for nn_DynamicSelectiveHyperNet.

Strategy
--------
Shard the target-parameter axis T across the 8 NeuronCores (no collectives
needed; the gated head-sum is computed locally per T-slice). Each core runs
all 8 heads for its slice:

  preamble (tiny, recomputed on every core):
    feats   = relu(x @ fe_W1.T + fe_b1) @ fe_W2.T + fe_b2          [8, 64]
    gate    = softmax(feats @ gate_W.T + gate_b, axis=1)           [8, 8]
    hin     = concat(feats[b], embeds[p])                          [32, 96]
    hmid[h] = relu(hin @ gen_W1[h].T + gen_b1[h])                  [32, 32]
  main loop over heads x T-chunks (streamed from HBM):
    imp  = sigmoid(hin @ att_W[h].T + att_b[h])      K=96 (+1 bias row)
    gw   = gate[h,b] * (hmid[h] @ gen_W2[h].T + gen_b2[h])  K=32 (+1 row)
    acc += imp * gw

Big weights are passed pre-transposed ([K, T] layout, contraction index on
SBUF partitions) with the bias appended as one extra contraction row against
a constant-one row in the stationary operand. The gate factor (including the
softmax normalization) is folded into the gen stationary operand. Matmuls
use 4-way PE column tiling so PSUM/DVE tiles are a full 128 partitions.
"""

import sys

sys.path.insert(0, "/opt/trn_rl_repo")

import json

import numpy as np

import concourse.bass as bass
import concourse.bass2jax as _bass2jax
import concourse.bass_utils as _bass_utils
import concourse.tile as tile
from concourse import mybir
from concourse.bass_utils import run_bass_kernel_spmd

AF = mybir.ActivationFunctionType
ALU = mybir.AluOpType
F32 = mybir.dt.float32
AX = mybir.AxisListType

B = 8
H = 8
NP = 4          # target param groups
FEAT = 64
EMB = 32
HIN = 96        # FEAT + EMB
GH = 32         # generator hidden
T = 101770
NCORES = 8
TS = 12800      # per-core T shard (8*TS = 102400 >= T, zero padded)
SUP = 2048      # supertile columns (4 col-groups x 512)
NSUB = 512
KFE = 896       # 784 padded to 7*128
PB = NP * B     # 32

# ---------------------------------------------------------------------------
# Workaround: this container's walrus build rejects more than one sync-wait
# command per instruction, while Tile freely attaches several. Split the
# extra waits onto same-engine NoOps inserted just before the instruction
# (same semantics: the engine's sequencer blocks on each wait in order).
# ---------------------------------------------------------------------------
_orig_compile_bir_kernel = _bass_utils.compile_bir_kernel


def _split_multi_waits(bir):
    for fn in bir.get("functions", []):
        for bb in fn.get("blocks", []):
            out = []
            for ins in bb.get("instructions", []):
                si = ins.get("sync_info")
                waits = (si or {}).get("on_wait") or []
                if len(waits) > 1:
                    for k, w in enumerate(waits[:-1]):
                        out.append({
                            "debug": ins.get("debug", 0),
                            "engine": ins["engine"],
                            "ins": [],
                            "name": f"{ins['name']}-wsplit{k}",
                            "opcode": "NoOp",
                            "outs": [],
                            "sync_info": {"on_update": [], "on_wait": [w]},
                        })
                    si["on_wait"] = [waits[-1]]
                out.append(ins)
            bb["instructions"] = out
    return bir


def _patched_compile_bir_kernel(bir_json, tmpdir, neff_name="file.neff"):
    bir = _split_multi_waits(json.loads(bir_json))
    return _orig_compile_bir_kernel(json.dumps(bir).encode(), tmpdir,
                                    neff_name=neff_name)


def _install_patch():
    _bass_utils.compile_bir_kernel = _patched_compile_bir_kernel
    _bass2jax.compile_bir_kernel = _patched_compile_bir_kernel


_install_patch()


# ---------------------------------------------------------------------------
# Device program
# ---------------------------------------------------------------------------
def _build_bass(ts=TS):
    nc = bass.Bass()

    att_in = nc.dram_tensor("att_in", [H, HIN + 1, ts], F32, kind="ExternalInput")
    gen_in = nc.dram_tensor("gen_in", [H, GH + 1, ts], F32, kind="ExternalInput")
    xt = nc.dram_tensor("xt", [KFE, B], F32, kind="ExternalInput")
    fe1t = nc.dram_tensor("fe1t", [KFE, 128], F32, kind="ExternalInput")
    fb1 = nc.dram_tensor("fb1", [128, 1], F32, kind="ExternalInput")
    fw2t = nc.dram_tensor("fw2t", [128, FEAT], F32, kind="ExternalInput")
    fb2 = nc.dram_tensor("fb2", [FEAT, 1], F32, kind="ExternalInput")
    gwt = nc.dram_tensor("gwt", [FEAT + 1, H], F32, kind="ExternalInput")
    emb = nc.dram_tensor("emb", [EMB, PB], F32, kind="ExternalInput")
    sel4 = nc.dram_tensor("sel4", [B, PB], F32, kind="ExternalInput")
    g1in = nc.dram_tensor("g1in", [HIN + 1, H * GH], F32, kind="ExternalInput")
    out = nc.dram_tensor("out", [PB, ts], F32, kind="ExternalOutput")

    n_sup = ts // SUP  # full supertiles; plus one 512-wide tail
    assert ts == n_sup * SUP + NSUB

    with tile.TileContext(nc) as tc:
        with (
            tc.tile_pool(name="const", bufs=1) as cp,
            tc.tile_pool(name="stream", bufs=4) as sp,
            tc.tile_pool(name="psum", bufs=2, space="PSUM") as pp,
            tc.tile_pool(name="prepsum", bufs=1, space="PSUM") as prep,
            tc.tile_pool(name="ev", bufs=3) as ev,
            tc.tile_pool(name="accp", bufs=2) as accp,
        ):
            # ---- constant loads -------------------------------------------
            fe1_t = cp.tile([128, 7, 128], F32)
            nc.sync.dma_start(fe1_t[:], fe1t.rearrange("(o p) m -> p o m", p=128))
            xt_t = cp.tile([128, 7, B], F32)
            nc.sync.dma_start(xt_t[:], xt.rearrange("(o p) m -> p o m", p=128))
            fb1_t = cp.tile([128, 1], F32)
            nc.sync.dma_start(fb1_t[:], fb1[:])
            fw2_t = cp.tile([128, FEAT], F32)
            nc.sync.dma_start(fw2_t[:], fw2t[:])
            fb2_t = cp.tile([FEAT, 1], F32)
            nc.sync.dma_start(fb2_t[:], fb2[:])
            gwt_t = cp.tile([FEAT + 1, H], F32)
            nc.sync.dma_start(gwt_t[:], gwt[:])
            sel4_t = cp.tile([B, PB], F32)
            nc.sync.dma_start(sel4_t[:], sel4[:])
            g1_t = cp.tile([HIN + 1, H * GH], F32)
            nc.sync.dma_start(g1_t[:], g1in[:])

            hinT = cp.tile([HIN + 1, PB], F32)      # [97, 32] stationary (att)
            lgen = cp.tile([GH + 1, H * PB], F32)   # [33, 8*32] stationary (gen)

            # ---- feature extractor ----------------------------------------
            psf = prep.tile([128, 32], F32, tag="pre1")
            for o in range(7):
                nc.tensor.matmul(psf[:, :B], fe1_t[:, o, :], xt_t[:, o, :],
                                 start=(o == 0), stop=(o == 6))
            relu1 = cp.tile([128, B], F32)
            nc.scalar.activation(relu1[:], psf[:, :B], AF.Relu, bias=fb1_t[:])

            psf2 = prep.tile([128, 32], F32, tag="pre2")
            nc.tensor.matmul(psf2[:FEAT, :B], fw2_t[:], relu1[:],
                             start=True, stop=True)
            featsT = cp.tile([FEAT + 1, B], F32)    # [65, 8], row 64 = ones
            nc.scalar.activation(featsT[:FEAT, :], psf2[:FEAT, :B], AF.Identity,
                                 bias=fb2_t[:])
            nc.vector.memset(featsT[FEAT:FEAT + 1, :], 1.0)

            # ---- head gate (softmax over heads, normalization folded) -----
            psgl = prep.tile([128, 32], F32, tag="pre1")
            nc.tensor.matmul(psgl[:B, :B], featsT[:], gwt_t[:],
                             start=True, stop=True)
            gateb = cp.tile([32, 32], F32)          # gate[b, h] in [0:8, 0:8]
            nc.vector.memset(gateb[:], 0.0)
            nc.scalar.activation(gateb[:B, :B], psgl[:B, :B], AF.Exp)
            sums = cp.tile([B, 1], F32)
            nc.vector.tensor_reduce(sums[:], gateb[:B, :B], AX.X, ALU.add)
            recip = cp.tile([B, 1], F32)
            nc.vector.reciprocal(recip[:], sums[:])
            nc.vector.tensor_scalar_mul(gateb[:B, :B], gateb[:B, :B], recip[:])
            gatebT = cp.tile([32, 32], F32)         # gate[h, b] in [0:8, 0:8]
            nc.vector.transpose(gatebT[:], gateb[:])
            # gate column per (pb, h): gcols[pb, h] = gate[h, pb % 8]
            psgc = prep.tile([128, 32], F32, tag="pre1")
            nc.tensor.matmul(psgc[:PB, :B], sel4_t[:], gatebT[:B, :B],
                             start=True, stop=True)
            gcols = cp.tile([PB, B], F32)
            nc.vector.tensor_copy(gcols[:], psgc[:PB, :B])

            # ---- hin (stationary operand of the att matmuls) --------------
            for p in range(NP):
                nc.vector.tensor_copy(hinT[:FEAT, p * B:(p + 1) * B],
                                      featsT[:FEAT, :])
            nc.sync.dma_start(hinT[FEAT:HIN, :], emb[:])
            nc.vector.memset(hinT[HIN:HIN + 1, :], 1.0)

            # ---- per-head gen stationary operand --------------------------
            for h in range(H):
                psh = prep.tile([128, 32], F32, tag="preh")
                nc.tensor.matmul(psh[:PB, :GH], hinT[:], g1_t[:, h * GH:(h + 1) * GH],
                                 start=True, stop=True)
                hmid = cp.tile([PB, GH], F32, tag="hmid")
                nc.scalar.activation(hmid[:], psh[:PB, :GH], AF.Relu)
                nc.vector.tensor_scalar_mul(hmid[:], hmid[:], gcols[:, h:h + 1])
                nc.vector.transpose(lgen[:GH, h * PB:(h + 1) * PB], hmid[:])
                nc.tensor.matmul(psh[GH:GH + 1, :PB], gatebT[:B, h:h + 1],
                                 sel4_t[:], start=True, stop=True,
                                 tile_position=(0, 32))
                nc.vector.tensor_copy(lgen[GH:GH + 1, h * PB:(h + 1) * PB],
                                      psh[GH:GH + 1, :PB])

            # ---- main streamed loop ---------------------------------------
            for s in range(n_sup + 1):
                ncols = SUP if s < n_sup else NSUB
                ns = ncols // 4
                c0 = s * SUP
                acc = accp.tile([128, NSUB], F32, tag="acc")
                for h in range(H):
                    att_t = sp.tile([HIN + 1, SUP], F32, tag="att")
                    nc.sync.dma_start(att_t[:, :ncols],
                                      att_in[h, :, c0:c0 + ncols])
                    gen_t = sp.tile([GH + 1, SUP], F32, tag="gen")
                    nc.sync.dma_start(gen_t[:, :ncols],
                                      gen_in[h, :, c0:c0 + ncols])
                    psA = pp.tile([128, NSUB], F32, tag="psA")
                    psG = pp.tile([128, NSUB], F32, tag="psG")
                    for g in range(4):
                        nc.tensor.matmul(psA[32 * g:32 * (g + 1), :ns],
                                         hinT[:], att_t[:, g * ns:(g + 1) * ns],
                                         start=True, stop=True,
                                         tile_position=(0, 32 * g))
                    for g in range(4):
                        nc.tensor.matmul(psG[32 * g:32 * (g + 1), :ns],
                                         lgen[:, h * PB:(h + 1) * PB],
                                         gen_t[:, g * ns:(g + 1) * ns],
                                         start=True, stop=True,
                                         tile_position=(0, 32 * g))
                    imp = ev.tile([128, NSUB], F32, tag="imp")
                    nc.scalar.activation(imp[:, :ns], psA[:, :ns], AF.Sigmoid)
                    if h == 0:
                        nc.vector.tensor_tensor(acc[:, :ns], imp[:, :ns],
                                                psG[:, :ns], ALU.mult)
                    else:
                        tmp = ev.tile([128, NSUB], F32, tag="tmp")
                        nc.vector.tensor_tensor(tmp[:, :ns], imp[:, :ns],
                                                psG[:, :ns], ALU.mult)
                        nc.vector.tensor_add(acc[:, :ns], acc[:, :ns],
                                             tmp[:, :ns])
                nc.sync.dma_start(
                    out[:, c0:c0 + ncols].rearrange("p (g c) -> g p c", g=4),
                    acc[:, :ns])
    return nc


_NC_CACHE = None


def _get_nc():
    global _NC_CACHE
    if _NC_CACHE is None:
        _NC_CACHE = _build_bass()
    return _NC_CACHE


# ---------------------------------------------------------------------------
# Host wrapper
# ---------------------------------------------------------------------------
LAST_RESULTS = None  # BassKernelResults of the last run (for profiling)
LAST_IN_MAPS = None  # per-core input maps of the last run (for benchmarking)


def kernel(x, fe_W1, fe_b1, fe_W2, fe_b2, embeds,
           gen_W1, gen_b1, gen_W2, gen_b2, att_W, att_b,
           gate_W, gate_b):
    import os

    f32 = np.float32
    x = np.asarray(x, f32)
    fe_W1 = np.asarray(fe_W1, f32)
    fe_b1 = np.asarray(fe_b1, f32)
    fe_W2 = np.asarray(fe_W2, f32)
    fe_b2 = np.asarray(fe_b2, f32)
    embeds = np.asarray(embeds, f32)
    gen_W1 = np.asarray(gen_W1, f32)
    gen_b1 = np.asarray(gen_b1, f32)
    gen_W2 = np.asarray(gen_W2, f32)
    gen_b2 = np.asarray(gen_b2, f32)
    att_W = np.asarray(att_W, f32)
    att_b = np.asarray(att_b, f32)
    gate_W = np.asarray(gate_W, f32)
    gate_b = np.asarray(gate_b, f32)

    # --- big streamed operands: [H, K+1, T_pad] with bias as extra row ---
    tpad = NCORES * TS
    att_all = np.zeros((H, HIN + 1, tpad), f32)
    att_all[:, :HIN, :T] = att_W.transpose(0, 2, 1)
    att_all[:, HIN, :T] = att_b
    gen_all = np.zeros((H, GH + 1, tpad), f32)
    gen_all[:, :GH, :T] = gen_W2.transpose(0, 2, 1)
    gen_all[:, GH, :T] = gen_b2

    # --- small shared operands ---
    xt = np.zeros((KFE, B), f32)
    xt[:784] = x.T
    fe1t = np.zeros((KFE, 128), f32)
    fe1t[:784] = fe_W1.T
    fb1 = np.ascontiguousarray(fe_b1[:, None])
    fw2t = np.ascontiguousarray(fe_W2.T)
    fb2 = np.ascontiguousarray(fe_b2[:, None])
    gwt = np.concatenate([gate_W.T, gate_b[None, :]], axis=0)
    emb = np.repeat(embeds.T[:, :, None], B, axis=2).reshape(EMB, PB)
    sel4 = np.tile(np.eye(B, dtype=f32), NP)
    g1in = np.concatenate([gen_W1.transpose(0, 2, 1), gen_b1[:, None, :]],
                          axis=1)                      # [H, 97, 32]
    g1in = g1in.transpose(1, 0, 2).reshape(HIN + 1, H * GH)

    shared = {
        "xt": xt, "fe1t": fe1t, "fb1": fb1, "fw2t": fw2t, "fb2": fb2,
        "gwt": np.ascontiguousarray(gwt), "emb": np.ascontiguousarray(emb),
        "sel4": np.ascontiguousarray(sel4), "g1in": np.ascontiguousarray(g1in),
    }
    in_maps = []
    for c in range(NCORES):
        sl = slice(c * TS, (c + 1) * TS)
        m = dict(shared)
        m["att_in"] = np.ascontiguousarray(att_all[:, :, sl])
        m["gen_in"] = np.ascontiguousarray(gen_all[:, :, sl])
        in_maps.append(m)

    nc = _get_nc()
    res = run_bass_kernel_spmd(nc, in_maps, core_ids=list(range(NCORES)))
    global LAST_RESULTS, LAST_IN_MAPS
    LAST_RESULTS = res
    LAST_IN_MAPS = in_maps

    full = np.concatenate([res.results[c]["out"] for c in range(NCORES)],
                          axis=1)[:, :T]              # [32, T], row = p*8+b
    return np.ascontiguousarray(
        full.reshape(NP, B, T).transpose(1, 0, 2).reshape(B, NP * T))


# ---------------------------------------------------------------------------
# Timing harness (test-only): device-resident inputs, repeated execution.
# Mirrors bass2jax.run_bass_via_pjrt's multi-core path so only the NEFF
# execution (plus per-call dispatch and the small donated output buffers)
# is inside the timed region.
# ---------------------------------------------------------------------------
def benchmark_last(in_maps, iters=8):
    import time

    import jax
    from concourse import bass2jax as b2j
    from concourse import mybir as _mybir

    nc = _get_nc()
    b2j.install_neuronx_cc_hook()

    partition_name = (nc.partition_id_tensor.name
                      if nc.partition_id_tensor else None)
    in_names, out_names, out_avals, zero_outs = [], [], [], []
    for alloc in nc.m.functions[0].allocations:
        if not isinstance(alloc, _mybir.MemoryLocationSet):
            continue
        name = alloc.memorylocations[0].name
        if alloc.kind == "ExternalInput":
            if name != partition_name:
                in_names.append(name)
        elif alloc.kind == "ExternalOutput":
            shape = tuple(alloc.tensor_shape)
            dtype = _mybir.dt.np(alloc.dtype)
            out_names.append(name)
            out_avals.append(jax.core.ShapedArray(shape, dtype))
            zero_outs.append(np.zeros(shape, dtype))
    n_params = len(in_names)
    n_outs = len(out_avals)
    in_names_all = in_names + out_names
    if partition_name is not None:
        in_names_all.append(partition_name)

    def _body(*args):
        operands = list(args)
        if partition_name is not None:
            operands.append(b2j.partition_id_tensor())
        return tuple(b2j._bass_exec_p.bind(
            *operands,
            out_avals=tuple(out_avals),
            in_names=tuple(in_names_all),
            out_names=tuple(out_names),
            lowering_input_output_aliases=(),
            sim_require_finite=True,
            sim_require_nnan=True,
            nc=nc,
        ))

    donate = tuple(range(n_params, n_params + n_outs))
    devices = jax.devices()[:NCORES]
    mesh = b2j.Mesh(np.asarray(devices), ("core",))
    sharded = jax.jit(
        b2j.shard_map(_body, mesh=mesh,
                      in_specs=(b2j.PartitionSpec("core"),) * (n_params + n_outs),
                      out_specs=(b2j.PartitionSpec("core"),) * n_outs,
                      check_rep=False),
        donate_argnums=donate, keep_unused=True)

    concat_in = [
        np.concatenate([np.asarray(in_maps[c][nm]) for c in range(NCORES)],
                       axis=0)
        for nm in in_names
    ]
    sharding = jax.sharding.NamedSharding(mesh, b2j.PartitionSpec("core"))
    dev_in = [jax.device_put(a, sharding) for a in concat_in]

    def _zeros():
        return [jax.device_put(
            np.zeros((NCORES * z.shape[0], *z.shape[1:]), z.dtype), sharding)
            for z in zero_outs]

    # warmup (compile + load)
    outs = sharded(*dev_in, *_zeros())
    jax.block_until_ready(outs)
    times = []
    for _ in range(iters):
        zs = _zeros()
        jax.block_until_ready(zs)
        t0 = time.perf_counter()
        outs = sharded(*dev_in, *zs)
        jax.block_until_ready(outs)
        times.append(time.perf_counter() - t0)
    return min(times), times
